# revision 1
# baseline (speedup 1.0000x reference)
"""Two-layer residual GCN (PyG GCNConv-style) on 8 Trainium2 NeuronCores.

Strategy (per the dst-sharding plan):
  - Nodes are sharded across the 8 cores (rows of x). Edges are partitioned
    by destination shard so the scatter-add (segment_sum) is local to a core.
  - Per layer, each core computes g = (h @ W) * dinv for its node shard, the
    shards are AllGather'ed so every core holds the full "message table",
    and each core then gathers source rows of that table (dma_gather) and
    scatter-adds them into its destination windows with one-hot matmuls on
    the tensor engine (PSUM accumulation per 128-node window).
  - dinv = 1/sqrt(deg) is computed on-device from CSR row pointers
    (deg = rowptrB - rowptrA, including self-loops).
  - Self-loop edges are materialized as explicit edges (src == dst), so the
    aggregation needs no special casing.

The host side only reorders / partitions data (sorting edges by destination,
CSR construction, padding) — all arithmetic runs on the NeuronCores.
"""

import math
import os
import sys

import numpy as np

for _p in ("/opt/trn_rl_repo",):
    if _p not in sys.path and os.path.isdir(_p):
        sys.path.insert(0, _p)

from concourse import bacc, bass, mybir
from concourse.tile import TileContext

F32 = mybir.dt.float32
I16 = mybir.dt.int16

N_CORES = 8
N_CHUNKS = 4  # quarters of each shard; chunk table rows = 8 * qrows <= 32767
GROUP_W = 4  # windows per gather group
D = 128


# --------------------------------------------------------------------------
# Planning (host): shapes, edge partition, paddings
# --------------------------------------------------------------------------
class Plan:
    def __init__(self, n_nodes, n_edges):
        self.N = n_nodes
        self.E = n_edges
        self.shard = -(-n_nodes // N_CORES)  # real rows per core (last may be short)
        self.qrows = -(-(-(-self.shard // N_CHUNKS)) // 128) * 128
        self.SH = N_CHUNKS * self.qrows  # padded shard rows
        self.W = self.SH // 128  # windows per core
        self.WQ = self.qrows // 128  # windows per quarter
        self.TROWS = N_CORES * self.qrows  # rows per chunk table
        assert self.TROWS <= 32767, "chunk table must be int16-indexable"
        # groups: per quarter, windows in chunks of GROUP_W
        self.groups = []  # list of (q, [w, ...])
        for q in range(N_CHUNKS):
            ws = list(range(q * self.WQ, (q + 1) * self.WQ))
            for i in range(0, len(ws), GROUP_W):
                self.groups.append((q, ws[i : i + GROUP_W]))


def _host_prepare(x, edge_index, W1, b1, W2, b2):
    """Shard + sort the graph; build all per-core input arrays."""
    N, d = x.shape
    assert d == D
    E = edge_index.shape[1]
    p = Plan(N, E)

    src = np.ascontiguousarray(edge_index[0]).astype(np.int64)
    dst = np.ascontiguousarray(edge_index[1]).astype(np.int64)
    loop = np.arange(N, dtype=np.int64)
    s_all = np.concatenate([src, loop])
    d_all = np.concatenate([dst, loop])

    core = d_all // p.shard
    l_dst = d_all - core * p.shard  # local dst row
    win = l_dst // 128
    # source -> (chunk, table row)
    r_src = s_all // p.shard
    l_src = s_all - r_src * p.shard
    q_src = l_src // p.qrows
    t_row = r_src * p.qrows + (l_src - q_src * p.qrows)

    # per-core cell counts [core, chunk, window]
    ncell = N_CHUNKS * p.W
    cellid = q_src * p.W + win
    counts = np.zeros((N_CORES, N_CHUNKS, p.W), dtype=np.int64)
    order_all = []
    for c in range(N_CORES):
        m = core == c
        cid = cellid[m]
        counts[c] = np.bincount(cid, minlength=ncell).reshape(N_CHUNKS, p.W)
        o = np.argsort(cid, kind="stable")
        order_all.append((np.nonzero(m)[0][o], cid[o]))

    K = (-(-counts // 128)).max(axis=0)  # [chunk, W] blocks per cell (shared)

    # degree / CSR rowptr per core (over local dst, includes self-loops)
    rptrA = np.zeros((N_CORES, 128, p.W), dtype=np.float32)
    rptrB = np.zeros((N_CORES, 128, p.W), dtype=np.float32)

    # static program structure
    # per group g: list over chunks of (idx_col_off, ncols, [(w, K_kw, b0), ...])
    btot = 0
    totcols = 0
    struct = []
    for q, ws in p.groups:
        per_chunk = []
        for k in range(N_CHUNKS):
            cells = []
            nblk = 0
            for w in ws:
                kk = int(K[k, w])
                if kk > 0:
                    cells.append((w, kk, btot))
                    btot += kk
                    nblk += kk
            ncols = 8 * nblk  # int16 cols = 128*nblk/16
            per_chunk.append((totcols, nblk, cells))
            totcols += ncols
        struct.append(per_chunk)
    p.struct = struct
    p.BTOT = max(btot, 1)
    p.TOTC = max(totcols, 8)

    # fill per-core arrays
    idx16 = np.zeros((N_CORES, 16, p.TOTC), dtype=np.int16)
    dstrel = np.full((N_CORES, 128, p.BTOT), -1.0, dtype=np.float32)
    for c in range(N_CORES):
        eidx, cid = order_all[c]
        cell_starts = np.zeros(ncell + 1, dtype=np.int64)
        np.cumsum(np.bincount(cid, minlength=ncell), out=cell_starts[1:])
        tr = t_row[eidx]
        dr = (l_dst[eidx] % 128).astype(np.float32)
        for (q, ws), per_chunk in zip(p.groups, struct):
            for k in range(N_CHUNKS):
                col0, nblk, cells = per_chunk[k]
                if nblk == 0:
                    continue
                seg = np.zeros(128 * nblk, dtype=np.int16)
                segoff = 0
                for w, kk, b0 in cells:
                    s0, s1 = cell_starts[k * p.W + w], cell_starts[k * p.W + w + 1]
                    cnt = s1 - s0
                    pad = kk * 128
                    vals = np.zeros(pad, dtype=np.int16)
                    vals[:cnt] = tr[s0:s1].astype(np.int16)
                    seg[segoff : segoff + pad] = vals
                    drv = np.full(pad, -1.0, dtype=np.float32)
                    drv[:cnt] = dr[s0:s1]
                    dstrel[c, :, b0 : b0 + kk] = drv.reshape(kk, 128).T
                    segoff += pad
                idx16[c, :, col0 : col0 + 8 * nblk] = seg.reshape(-1, 16).T

        # rowptr (local dst sorted counts) including self-loops
        degc = np.bincount(l_dst[core == c], minlength=p.SH).astype(np.int64)
        nreal = min(N - c * p.shard, p.shard)
        if nreal < p.SH:
            degc[nreal:] = 1  # pads: deg=1 -> dinv=1, x=0 -> harmless
        rp = np.zeros(p.SH + 1, dtype=np.int64)
        np.cumsum(degc, out=rp[1:])
        rptrA[c] = rp[:-1].reshape(p.W, 128).T.astype(np.float32)
        rptrB[c] = rp[1:].reshape(p.W, 128).T.astype(np.float32)

    idx128 = np.tile(idx16, (1, 8, 1))  # replicate across the 8 gpsimd cores

    # node features, padded + tiled
    x_pad = np.zeros((N_CORES, p.SH, D), dtype=np.float32)
    for c in range(N_CORES):
        n0 = c * p.shard
        nreal = max(0, min(N - n0, p.shard))
        if nreal > 0:
            x_pad[c, :nreal] = x[n0 : n0 + nreal]
    x_tiled = np.ascontiguousarray(
        x_pad.reshape(N_CORES, p.W, 128, D)
    )  # [c, w, p, f]
    xT = np.ascontiguousarray(x_pad.transpose(0, 2, 1))  # [c, D, SH]

    iota = np.tile(np.arange(128, dtype=np.float32), (128, 1))
    ident = np.eye(128, dtype=np.float32)
    b1t = np.tile(b1.astype(np.float32), (128, 1))
    b2t = np.tile(b2.astype(np.float32), (128, 1))

    in_maps = []
    for c in range(N_CORES):
        in_maps.append(
            {
                "x_tiled": x_tiled[c],
                "xT": xT[c],
                "W1": W1.astype(np.float32),
                "W2": W2.astype(np.float32),
                "b1t": b1t,
                "b2t": b2t,
                "iota": iota,
                "ident": ident,
                "rptrA": rptrA[c],
                "rptrB": rptrB[c],
                "idx16": idx128[c],
                "dstrel": dstrel[c],
            }
        )
    return p, in_maps


# --------------------------------------------------------------------------
# Device program
# --------------------------------------------------------------------------
def _build_program(p: Plan):
    from contextlib import ExitStack

    nc = bacc.Bacc(
        "TRN2", target_bir_lowering=False, debug=False, num_devices=N_CORES
    )
    RG = [list(range(N_CORES))]

    x_tiled = nc.dram_tensor("x_tiled", [p.W, 128, D], F32, kind="ExternalInput")
    xT = nc.dram_tensor("xT", [D, p.SH], F32, kind="ExternalInput")
    W1 = nc.dram_tensor("W1", [D, D], F32, kind="ExternalInput")
    W2 = nc.dram_tensor("W2", [D, D], F32, kind="ExternalInput")
    b1t = nc.dram_tensor("b1t", [128, D], F32, kind="ExternalInput")
    b2t = nc.dram_tensor("b2t", [128, D], F32, kind="ExternalInput")
    iota_d = nc.dram_tensor("iota", [128, 128], F32, kind="ExternalInput")
    ident_d = nc.dram_tensor("ident", [128, 128], F32, kind="ExternalInput")
    rptrA_d = nc.dram_tensor("rptrA", [128, p.W], F32, kind="ExternalInput")
    rptrB_d = nc.dram_tensor("rptrB", [128, p.W], F32, kind="ExternalInput")
    idx_d = nc.dram_tensor("idx16", [128, p.TOTC], I16, kind="ExternalInput")
    dstrel_d = nc.dram_tensor("dstrel", [128, p.BTOT], F32, kind="ExternalInput")

    out_d = nc.dram_tensor("out", [p.W, 128, D], F32, kind="ExternalOutput")

    # internal DRAM
    gq = [
        [nc.dram_tensor(f"g{layer}q{q}", [p.qrows, D], F32) for q in range(N_CHUNKS)]
        for layer in (1, 2)
    ]
    tables = [
        [
            nc.dram_tensor(f"t{layer}q{q}", [p.TROWS, D], F32, addr_space="Shared")
            for q in range(N_CHUNKS)
        ]
        for layer in (1, 2)
    ]
    h1_d = nc.dram_tensor("h1", [p.W, 128, D], F32)

    with TileContext(nc) as tc:
        ctx = ExitStack()
        cst = ctx.enter_context(tc.tile_pool(name="cst", bufs=1))
        w1_sb = cst.tile([D, D], F32, tag="w1")
        w2_sb = cst.tile([D, D], F32, tag="w2")
        b1_sb = cst.tile([128, D], F32, tag="b1")
        b2_sb = cst.tile([128, D], F32, tag="b2")
        iota_sb = cst.tile([128, 128], F32, tag="iota")
        ident_sb = cst.tile([128, 128], F32, tag="ident")
        dinv_sb = cst.tile([128, p.W], F32, tag="dinv")
        zero_sb = cst.tile([128, D], F32, tag="zero")
        nc.vector.memset(zero_sb[:, :], 0.0)
        dstrel_sb = cst.tile([128, p.BTOT], F32, tag="dstrel")
        for t, dr in (
            (w1_sb, W1),
            (w2_sb, W2),
            (b1_sb, b1t),
            (b2_sb, b2t),
            (iota_sb, iota_d),
            (ident_sb, ident_d),
            (dstrel_sb, dstrel_d),
        ):
            nc.sync.dma_start(out=t[:, :], in_=dr[:, :])

        # deg -> dinv = sqrt(1/deg)
        tmp_pool = ctx.enter_context(tc.tile_pool(name="deg", bufs=1))
        ra = tmp_pool.tile([128, p.W], F32, tag="ra")
        rb = tmp_pool.tile([128, p.W], F32, tag="rb")
        nc.sync.dma_start(out=ra[:, :], in_=rptrA_d[:, :])
        nc.sync.dma_start(out=rb[:, :], in_=rptrB_d[:, :])
        nc.vector.tensor_sub(dinv_sb[:, :], rb[:, :], ra[:, :])  # deg
        nc.vector.reciprocal(dinv_sb[:, :], dinv_sb[:, :])
        nc.scalar.sqrt(dinv_sb[:, :], dinv_sb[:, :])

        def g_phase(layer, src_mm):
            """g{layer}[w] = (src @ W) * dinv per window; DMA to quarter bufs,
            AllGather each quarter into its chunk table.
            src_mm(w) -> (lhsT_ap [128(fin) x 128(node)], rhs_sb)"""
            with (
                tc.tile_pool(name=f"gp{layer}", bufs=4) as gp,
                tc.tile_pool(name=f"gpsum{layer}", bufs=2, space="PSUM") as pp,
            ):
                for q in range(N_CHUNKS):
                    for wq in range(p.WQ):
                        w = q * p.WQ + wq
                        lhsT, rhs = src_mm(w)
                        ps = pp.tile([128, D], F32, tag="ps")
                        nc.tensor.matmul(
                            ps[:, :], lhsT, rhs, start=True, stop=True
                        )
                        gt = gp.tile([128, D], F32, tag="gt")
                        nc.vector.tensor_scalar(
                            gt[:, :],
                            ps[:, :],
                            dinv_sb[:, w : w + 1],
                            None,
                            mybir.AluOpType.mult,
                        )
                        nc.sync.dma_start(
                            out=gq[layer - 1][q][wq * 128 : (wq + 1) * 128, :],
                            in_=gt[:, :],
                        )
                    nc.gpsimd.collective_compute(
                        "AllGather",
                        mybir.AluOpType.bypass,
                        replica_groups=RG,
                        ins=[gq[layer - 1][q][:, :]],
                        outs=[tables[layer - 1][q][:, :]],
                    )

        def agg_phase(layer, epilogue):
            """Gather + one-hot matmul scatter into per-window PSUM, then
            epilogue(w, psum_tile, pools)."""
            table = tables[layer - 1]
            with (
                tc.tile_pool(name=f"msg{layer}", bufs=2) as mp,
                tc.tile_pool(name=f"amat{layer}", bufs=2) as ap_,
                tc.tile_pool(name=f"idx{layer}", bufs=3) as ip,
                tc.tile_pool(name=f"apsum{layer}", bufs=6, space="PSUM") as pp,
                tc.tile_pool(name=f"epi{layer}", bufs=4) as ep,
                tc.tile_pool(name=f"epsum{layer}", bufs=2, space="PSUM") as p2,
            ):
                for (q, ws), per_chunk in zip(p.groups, p.struct):
                    ws = [w for w in ws if w * 128 < p.shard]
                    if not ws:
                        continue
                    psums = {w: pp.tile([128, D], F32, tag="ps", name=f"ps{w}") for w in ws}
                    first = {w: True for w in ws}
                    last = {}
                    for k in range(N_CHUNKS):
                        for w, kk, b0 in per_chunk[k][2]:
                            last[w] = (k, b0 + kk - 1)
                    for k in range(N_CHUNKS):
                        col0, nblk, cells = per_chunk[k]
                        if nblk == 0:
                            continue
                        nidx = 128 * nblk
                        it = ip.tile([128, 8 * nblk], I16, tag="it")
                        nc.sync.dma_start(
                            out=it[:, :], in_=idx_d[:, col0 : col0 + 8 * nblk]
                        )
                        mt = mp.tile([128, nblk, D], F32, tag="mt")
                        nc.gpsimd.dma_gather(
                            out_ap=mt[:, :, :],
                            in_ap=table[k][:, :],
                            idxs_ap=it[:, :],
                            num_idxs=nidx,
                            num_idxs_reg=nidx,
                            elem_size=D,
                            single_packet=False,
                        )
                        # build all one-hot blocks for this (group, chunk)
                        nb0 = cells[0][2]
                        at = ap_.tile([128, nblk, 128], F32, tag="at")
                        nc.vector.tensor_tensor(
                            at[:, :, :],
                            iota_sb.unsqueeze(1).broadcast_to([128, nblk, 128]),
                            dstrel_sb[:, nb0 : nb0 + nblk]
                            .unsqueeze(2)
                            .broadcast_to([128, nblk, 128]),
                            mybir.AluOpType.is_equal,
                        )
                        for w, kk, b0 in cells:
                            for j in range(kk):
                                jb = b0 - nb0 + j
                                st = first[w]
                                first[w] = False
                                sp = last[w] == (k, b0 + j)
                                nc.tensor.matmul(
                                    psums[w][:, :],
                                    at[:, jb, :],
                                    mt[:, jb, :],
                                    start=st,
                                    stop=sp,
                                )
                    for w in ws:
                        epilogue(w, psums[w], ep, p2)

        # ---- layer 1 ----
        with tc.tile_pool(name="xT", bufs=1) as xT_pool:
            xT_sb = xT_pool.tile([D, p.SH], F32, tag="xT")
            nc.sync.dma_start(out=xT_sb[:, :], in_=xT[:, :])
            g_phase(1, lambda w: (xT_sb[:, w * 128 : (w + 1) * 128], w1_sb[:, :]))

            def epi1(w, ps, ep, p2):
                q = w // p.WQ
                wq = w % p.WQ
                xw = ep.tile([128, D], F32, tag="xw")
                nc.sync.dma_start(out=xw[:, :], in_=x_tiled[w, :, :])
                t1 = ep.tile([128, D], F32, tag="t1")
                nc.vector.tensor_scalar(
                    t1[:, :], ps[:, :], dinv_sb[:, w : w + 1], None,
                    mybir.AluOpType.mult,
                )
                nc.vector.tensor_add(t1[:, :], t1[:, :], xw[:, :])
                nc.vector.tensor_add(t1[:, :], t1[:, :], b1_sb[:, :])
                h1w = ep.tile([128, D], F32, tag="h1w")
                nc.scalar.activation(
                    h1w[:, :], t1[:, :], mybir.ActivationFunctionType.Relu
                )
                nc.sync.dma_start(out=h1_d[w, :, :], in_=h1w[:, :])
                # g2 for this window: transpose h1 then matmul W2, scale
                pt = p2.tile([128, D], F32, tag="pt")
                nc.tensor.transpose(pt[:, :], h1w[:, :], ident_sb[:, :])
                h1T = ep.tile([128, D], F32, tag="h1T")
                nc.vector.tensor_copy(h1T[:, :], pt[:, :])
                pg = p2.tile([128, D], F32, tag="pt")
                nc.tensor.matmul(
                    pg[:, :], h1T[:, :], w2_sb[:, :], start=True, stop=True
                )
                g2t = ep.tile([128, D], F32, tag="g2t")
                nc.vector.tensor_scalar(
                    g2t[:, :], pg[:, :], dinv_sb[:, w : w + 1], None,
                    mybir.AluOpType.mult,
                )
                nc.sync.dma_start(
                    out=gq[1][q][wq * 128 : (wq + 1) * 128, :], in_=g2t[:, :]
                )

            agg_phase(1, epi1)

        # pad-only windows never run the layer-1 epilogue: zero-fill their
        # g2 rows (the quarter AllGather reads the whole buffer) and output
        for w in range(p.W):
            if w * 128 < p.shard:
                continue
            nc.sync.dma_start(
                out=gq[1][w // p.WQ][(w % p.WQ) * 128 : (w % p.WQ + 1) * 128, :],
                in_=zero_sb[:, :],
            )
            nc.sync.dma_start(out=out_d[w, :, :], in_=zero_sb[:, :])

        # AllGather g2 quarters
        for q in range(N_CHUNKS):
            nc.gpsimd.collective_compute(
                "AllGather",
                mybir.AluOpType.bypass,
                replica_groups=RG,
                ins=[gq[1][q][:, :]],
                outs=[tables[1][q][:, :]],
            )

        # ---- layer 2 ----
        def epi2(w, ps, ep, p2):
            h1w = ep.tile([128, D], F32, tag="xw")
            nc.sync.dma_start(out=h1w[:, :], in_=h1_d[w, :, :])
            t1 = ep.tile([128, D], F32, tag="t1")
            nc.vector.tensor_scalar(
                t1[:, :], ps[:, :], dinv_sb[:, w : w + 1], None,
                mybir.AluOpType.mult,
            )
            nc.vector.tensor_add(t1[:, :], t1[:, :], h1w[:, :])
            nc.vector.tensor_add(t1[:, :], t1[:, :], b2_sb[:, :])
            nc.sync.dma_start(out=out_d[w, :, :], in_=t1[:, :])

        agg_phase(2, epi2)
        ctx.close()

    nc.compile()
    return nc


# --------------------------------------------------------------------------
# Entry point
# --------------------------------------------------------------------------
def kernel(x, edge_index, W1, b1, W2, b2):
    x = np.asarray(x)
    edge_index = np.asarray(edge_index)
    in_dtype = edge_index.dtype
    N = x.shape[0]
    p, in_maps = _host_prepare(
        np.asarray(x, dtype=np.float32),
        edge_index,
        np.asarray(W1, dtype=np.float32),
        np.asarray(b1, dtype=np.float32),
        np.asarray(W2, dtype=np.float32),
        np.asarray(b2, dtype=np.float32),
    )
    nc = _build_program(p)

    if os.environ.get("GCN_SIM"):
        from concourse import bass_interp

        sim = bass_interp.MultiCoreSim(nc, N_CORES)
        for c in range(N_CORES):
            for k, v in in_maps[c].items():
                sim.cores[c].tensor(k)[:] = v
        sim.simulate(check_with_hw=False)
        outs = [sim.cores[c].mem_tensor("out") for c in range(N_CORES)]
    else:
        from concourse.bass_utils import run_bass_kernel_spmd

        res = run_bass_kernel_spmd(
            nc,
            in_maps,
            list(range(N_CORES)),
            trace=bool(os.environ.get("GCN_TRACE")),
        )
        kernel.last_result = res
        outs = [res.results[c]["out"] for c in range(N_CORES)]

    full = np.concatenate(
        [np.asarray(o).reshape(p.SH, D)[: min(p.shard, N - c * p.shard)]
         for c, o in enumerate(outs)],
        axis=0,
    )
    return full.astype(np.float32)



# revision 2
# speedup vs baseline: 1.3716x; 1.3716x over previous
"""Two-layer residual GCN (PyG GCNConv-style) on 8 Trainium2 NeuronCores, v2.

Same dst-sharded skeleton as v1, tuned around the real bottleneck (GpSimd
descriptor generation for dma_gather, ~8 ns per gathered row, serial):

  - Self-loops are not materialized as edges: the epilogue adds the own-window
    table row (g_raw[d]*dinv[d]) before the final dinv[d] scale, giving
    dinv[d]^2*g_raw[d] exactly. Saves ~3% of gather descriptors.
  - Gather slots are sized per (chunk, window) cell to the max edge count over
    the 8 cores (SPMD program is shared), not rounded up to 128 per cell:
    saves ~5% descriptors. Blocks that straddle a window boundary get one
    masked one-hot pass per extra window.
  - dma_gather instructions rotate across 4 SWDGE queues: desc-gen for the
    next gather is not blocked behind the previous gather's ring drain.
  - Gathers are issued chunk-major so no gather waits on a later chunk's
    AllGather; per-window sums accumulate in an SBUF fp32 tile.
  - Tables, gathered messages, one-hots, and matmuls are bf16 (halves gather
    DMA bytes and AllGather wire, 2x LDWEIGHTS); the residual path (x, h1,
    accumulators, output) stays fp32.
  - dinv = 1/sqrt(deg+1) comes from the host.
"""

import os
import sys

import numpy as np

for _p in ("/opt/trn_rl_repo",):
    if _p not in sys.path and os.path.isdir(_p):
        sys.path.insert(0, _p)

from concourse import bacc, bass, mybir
from concourse.tile import TileContext

F32 = mybir.dt.float32
BF16 = mybir.dt.bfloat16
I16 = mybir.dt.int16

N_CORES = 8
N_CHUNKS = 4
GROUP_W = 4  # windows per gather unit
D = 128
QROT = int(os.environ.get("GCN_QROT", "4"))
SCRATCH = int(os.environ.get("GCN_SCRATCH", "16384"))


# --------------------------------------------------------------------------
# Planning (host): common SPMD structure + per-core data
# --------------------------------------------------------------------------
class Plan:
    def __init__(self, n_nodes, n_edges):
        self.N = n_nodes
        self.E = n_edges
        self.shard = -(-n_nodes // N_CORES)
        self.qrows = -(-(-(-self.shard // N_CHUNKS)) // 128) * 128
        self.SH = N_CHUNKS * self.qrows
        self.W = self.SH // 128
        self.WQ = self.qrows // 128
        self.TROWS = N_CORES * self.qrows
        assert self.TROWS <= 32767
        self.real_w = -(-self.shard // 128)


def _host_prepare(x, edge_index, W1, b1, W2, b2):
    import ml_dtypes

    N, d = x.shape
    assert d == D
    E = edge_index.shape[1]
    p = Plan(N, E)

    src = np.ascontiguousarray(edge_index[0]).astype(np.int64)
    dst = np.ascontiguousarray(edge_index[1]).astype(np.int64)

    core = dst // p.shard
    l_dst = dst - core * p.shard
    win = l_dst // 128
    dr = (l_dst % 128).astype(np.int64)
    r_src = src // p.shard
    l_src = src - r_src * p.shard
    q_src = l_src // p.qrows
    t_row = r_src * p.qrows + (l_src - q_src * p.qrows)

    # cell = (chunk, window); common size = max over cores
    ncell = N_CHUNKS * p.W
    cellid = q_src * p.W + win
    counts = np.zeros((N_CORES, ncell), dtype=np.int64)
    for c in range(N_CORES):
        counts[c] = np.bincount(cellid[core == c], minlength=ncell)
    L = counts.max(axis=0).reshape(N_CHUNKS, p.W)  # [chunk, window]

    # common slot layout: units = (chunk q, group g of GROUP_W windows)
    n_groups = -(-p.W // GROUP_W)
    units = []  # (q, icol0, n_slot, nblk, passes, windows)
    # passes: list of (blk, w, pcol, start, stop)
    wv_all = {}  # unit -> per-slot window id (-1 pad)
    cell_off = np.zeros((N_CHUNKS, p.W), dtype=np.int64)  # slot offset of cell
    icol = 0
    pcol = 0
    for q in range(N_CHUNKS):
        for g in range(n_groups):
            ws = list(range(g * GROUP_W, min((g + 1) * GROUP_W, p.W)))
            lens = [int(L[q, w]) for w in ws]
            tot = int(sum(lens))
            if tot == 0:
                units.append(None)
                continue
            n_slot = -(-tot // 128) * 128
            wv = np.full(n_slot, -1, dtype=np.int64)
            off = 0
            for w, ln in zip(ws, lens):
                cell_off[q, w] = icol * 16 + off
                wv[off : off + ln] = w
                off += ln
            nblk = -(-n_slot // 128)
            tmp = []
            for b in range(nblk):
                bw = wv[b * 128 : (b + 1) * 128]
                for w in np.unique(bw[bw >= 0]):
                    tmp.append((b, int(w), pcol))
                    pcol += 1
            first, last = {}, {}
            for b, w, pc in tmp:
                if w not in first:
                    first[w] = pc
                last[w] = pc
            passes = [
                (b, w, pc, pc == first[w], pc == last[w]) for b, w, pc in tmp
            ]
            units.append((q, icol, n_slot, nblk, passes, ws))
            wv_all[(q, g)] = wv
            icol += n_slot // 16
    p.units = units
    p.n_groups = n_groups
    p.TOTC = max(icol, 1)
    p.PASST = max(pcol, 1)
    p.MAXBLK = max((u[3] for u in units if u), default=1)
    p.MAXPASS = max((len(u[4]) for u in units if u), default=1)

    # degree incl self-loop -> dinv
    deg = np.bincount(dst, minlength=N).astype(np.float64) + 1.0
    dinv_full = (1.0 / np.sqrt(deg)).astype(np.float32)

    iota = np.tile(np.arange(128, dtype=np.float32), (128, 1))
    ident_bf = np.eye(128, dtype=np.float32).astype(ml_dtypes.bfloat16)
    b1t = np.tile(b1.astype(np.float32), (128, 1))
    b2t = np.tile(b2.astype(np.float32), (128, 1))

    in_maps = []
    for c in range(N_CORES):
        m = core == c
        ec, ew, et, edr = cellid[m], win[m], t_row[m], dr[m]
        order = np.argsort(ec, kind="stable")
        ec, ew, et, edr = ec[order], ew[order], et[order], edr[order]
        # slot position: cell offset + rank within cell
        cstart = np.zeros(ncell + 1, dtype=np.int64)
        np.cumsum(np.bincount(ec, minlength=ncell), out=cstart[1:])
        rank = np.arange(len(ec)) - cstart[ec]
        qq = ec // p.W
        wwin = ec % p.W
        slot = cell_off[qq, wwin] + rank

        idx_flat = np.zeros(p.TOTC * 16, dtype=np.int16)
        drel_slot = np.full(p.TOTC * 16, -1.0, dtype=np.float32)
        idx_flat[slot] = et.astype(np.int16)
        drel_slot[slot] = edr.astype(np.float32)

        # idx wrapped [16, TOTC] then replicated x8
        idx16 = idx_flat.reshape(p.TOTC, 16).T
        idx128 = np.tile(idx16, (8, 1))

        # per-pass drel columns [128, PASST]: block slots masked to the pass's
        # window via the common window layout
        drel = np.full((128, p.PASST), -1.0, dtype=np.float32)
        for u in units:
            if u is None:
                continue
            q, ic0, n_slot, nblk, passes, ws = u
            g = (ws[0]) // GROUP_W
            wv = wv_all[(q, g)]
            base = ic0 * 16
            for b, w, pc, st, sp in passes:
                s0 = b * 128
                s1 = min((b + 1) * 128, n_slot)
                seg_w = wv[s0:s1]
                seg_d = drel_slot[base + s0 : base + s1]
                col = np.where(seg_w == w, seg_d, -1.0).astype(np.float32)
                full = np.full(128, -1.0, dtype=np.float32)
                full[: len(col)] = col
                drel[:, pc] = full

        n0 = c * p.shard
        nreal = max(0, min(N - n0, p.shard))
        x_pad = np.zeros((p.SH, D), dtype=np.float32)
        x_pad[:nreal] = x[n0 : n0 + nreal]
        x_tiled = np.ascontiguousarray(x_pad.reshape(p.W, 128, D))
        xT = np.ascontiguousarray(x_pad.T).astype(ml_dtypes.bfloat16)

        dinv_pad = np.ones(p.SH, dtype=np.float32)
        dinv_pad[:nreal] = dinv_full[n0 : n0 + nreal]
        dinv_t = np.ascontiguousarray(dinv_pad.reshape(p.W, 128).T)

        in_maps.append(
            {
                "x_tiled": x_tiled,
                "xT": xT,
                "W1": W1.astype(ml_dtypes.bfloat16),
                "W2": W2.astype(ml_dtypes.bfloat16),
                "b1t": b1t,
                "b2t": b2t,
                "iota": iota,
                "ident_bf": ident_bf,
                "dinv": dinv_t,
                "idx16": idx128,
                "dstrel": drel,
            }
        )
    return p, in_maps


# --------------------------------------------------------------------------
# Device program
# --------------------------------------------------------------------------
def _build_program(p: Plan):
    from contextlib import ExitStack

    nc = bacc.Bacc(
        "TRN2",
        target_bir_lowering=False,
        debug=False,
        num_devices=N_CORES,
        num_swdge_queues=max(QROT, 1),
        dynamic_dma_scratch_size=SCRATCH,
    )
    RG = [list(range(N_CORES))]

    x_tiled = nc.dram_tensor("x_tiled", [p.W, 128, D], F32, kind="ExternalInput")
    xT_d = nc.dram_tensor("xT", [D, p.SH], BF16, kind="ExternalInput")
    W1_d = nc.dram_tensor("W1", [D, D], BF16, kind="ExternalInput")
    W2_d = nc.dram_tensor("W2", [D, D], BF16, kind="ExternalInput")
    b1_d = nc.dram_tensor("b1t", [128, D], F32, kind="ExternalInput")
    b2_d = nc.dram_tensor("b2t", [128, D], F32, kind="ExternalInput")
    iota_d = nc.dram_tensor("iota", [128, 128], F32, kind="ExternalInput")
    identb_d = nc.dram_tensor("ident_bf", [128, 128], BF16, kind="ExternalInput")
    dinv_d = nc.dram_tensor("dinv", [128, p.W], F32, kind="ExternalInput")
    idx_d = nc.dram_tensor("idx16", [128, p.TOTC], I16, kind="ExternalInput")
    drel_d = nc.dram_tensor("dstrel", [128, p.PASST], F32, kind="ExternalInput")

    out_d = nc.dram_tensor("out", [p.W, 128, D], F32, kind="ExternalOutput")

    gq = [
        [nc.dram_tensor(f"g{l}q{q}", [p.qrows, D], BF16) for q in range(N_CHUNKS)]
        for l in (1, 2)
    ]
    tables = [
        [
            nc.dram_tensor(f"t{l}q{q}", [p.TROWS, D], BF16, addr_space="Shared")
            for q in range(N_CHUNKS)
        ]
        for l in (1, 2)
    ]
    h1_d = nc.dram_tensor("h1", [p.W, 128, D], F32)

    gq_i = 0  # rotating swdge queue counter

    with TileContext(nc) as tc:
        ctx = ExitStack()
        cst = ctx.enter_context(tc.tile_pool(name="cst", bufs=1))
        w1_sb = cst.tile([D, D], BF16, tag="w1")
        w2_sb = cst.tile([D, D], BF16, tag="w2")
        b1_sb = cst.tile([128, D], F32, tag="b1")
        b2_sb = cst.tile([128, D], F32, tag="b2")
        iota_sb = cst.tile([128, 128], F32, tag="iota")
        identb_sb = cst.tile([128, 128], BF16, tag="identb")
        dinv_sb = cst.tile([128, p.W], F32, tag="dinv")
        for t, dr_ in (
            (w1_sb, W1_d),
            (w2_sb, W2_d),
            (b1_sb, b1_d),
            (b2_sb, b2_d),
            (iota_sb, iota_d),
            (identb_sb, identb_d),
            (dinv_sb, dinv_d),
        ):
            nc.sync.dma_start(out=t[:, :], in_=dr_[:, :])

        accum_pool = ctx.enter_context(tc.tile_pool(name="acc", bufs=1))
        accum = accum_pool.tile([128, p.W * 128], F32, tag="accum")

        def g1_phase():
            with tc.tile_pool(name="xT", bufs=1) as xp, \
                 tc.tile_pool(name="g1o", bufs=4) as go, \
                 tc.tile_pool(name="g1p", bufs=2, space="PSUM") as gp:
                xT_sb = xp.tile([D, p.SH], BF16, tag="xT")
                nc.sync.dma_start(out=xT_sb[:, :], in_=xT_d[:, :])
                for q in range(N_CHUNKS):
                    for wq in range(p.WQ):
                        w = q * p.WQ + wq
                        ps = gp.tile([128, D], F32, tag="ps")
                        nc.tensor.matmul(
                            ps[:, :],
                            xT_sb[:, w * 128 : (w + 1) * 128],
                            w1_sb[:, :],
                            start=True,
                            stop=True,
                        )
                        gt = go.tile([128, D], BF16, tag="gt")
                        nc.vector.tensor_scalar(
                            gt[:, :], ps[:, :], dinv_sb[:, w : w + 1], None,
                            mybir.AluOpType.mult,
                        )
                        nc.sync.dma_start(
                            out=gq[0][q][wq * 128 : (wq + 1) * 128, :],
                            in_=gt[:, :],
                        )
                    nc.gpsimd.collective_compute(
                        "AllGather",
                        mybir.AluOpType.bypass,
                        replica_groups=RG,
                        ins=[gq[0][q][:, :]],
                        outs=[tables[0][q][:, :]],
                    )

        def agg_phase(layer):
            nonlocal gq_i
            table = tables[layer - 1]
            nc.vector.memset(accum[:, :], 0.0)
            with (
                tc.tile_pool(name=f"mt{layer}", bufs=2) as mp,
                tc.tile_pool(name=f"at{layer}", bufs=2) as ap_,
                tc.tile_pool(name=f"ix{layer}", bufs=3) as ip,
                tc.tile_pool(name=f"dl{layer}", bufs=3) as dp,
                tc.tile_pool(name=f"ps{layer}", bufs=6, space="PSUM") as pp,
            ):
                for u in p.units:
                    if u is None:
                        continue
                    q, ic0, n_slot, nblk, passes, ws = u
                    it = ip.tile([128, n_slot // 16], I16, tag="it")
                    nc.sync.dma_start(
                        out=it[:, :], in_=idx_d[:, ic0 : ic0 + n_slot // 16]
                    )
                    npass = len(passes)
                    dt_ = dp.tile([128, npass], F32, tag="dl")
                    p0 = passes[0][2]
                    nc.sync.dma_start(
                        out=dt_[:, :], in_=drel_d[:, p0 : p0 + npass]
                    )
                    mt = mp.tile([128, p.MAXBLK, 128], BF16, tag="mt")
                    nc.gpsimd.dma_gather(
                        out_ap=mt[:, :nblk, :],
                        in_ap=table[q][:, :],
                        idxs_ap=it[:, :],
                        num_idxs=n_slot,
                        num_idxs_reg=n_slot,
                        elem_size=D,
                        single_packet=False,
                        queue_num=gq_i % max(QROT, 1),
                    )
                    gq_i += 1
                    at = ap_.tile([128, npass, 128], BF16, tag="at")
                    nc.vector.tensor_tensor(
                        at[:, :, :],
                        iota_sb.unsqueeze(1).broadcast_to([128, npass, 128]),
                        dt_.unsqueeze(2).broadcast_to([128, npass, 128]),
                        mybir.AluOpType.is_equal,
                    )
                    psums = {}
                    for b, w, pc, st, sp in passes:
                        if w not in psums:
                            psums[w] = pp.tile(
                                [128, D], F32, tag="ps", name=f"ps{layer}_{q}_{w}"
                            )
                        nc.tensor.matmul(
                            psums[w][:, :],
                            at[:, pc - p0, :],
                            mt[:, b, :],
                            start=st,
                            stop=sp,
                        )
                    for w in sorted(psums):
                        nc.vector.tensor_add(
                            accum[:, w * 128 : (w + 1) * 128],
                            accum[:, w * 128 : (w + 1) * 128],
                            psums[w][:, :],
                        )

        def epi1():
            with (
                tc.tile_pool(name="e1", bufs=6) as ep,
                tc.tile_pool(name="e1p", bufs=4, space="PSUM") as p2,
            ):
                for w in range(p.real_w):
                    q, wq = w // p.WQ, w % p.WQ
                    own = ep.tile([128, D], BF16, tag="own")
                    nc.sync.dma_start(
                        out=own[:, :], in_=gq[0][q][wq * 128 : (wq + 1) * 128, :]
                    )
                    xw = ep.tile([128, D], F32, tag="xw")
                    nc.sync.dma_start(out=xw[:, :], in_=x_tiled[w, :, :])
                    t1 = ep.tile([128, D], F32, tag="t1")
                    nc.vector.tensor_add(
                        t1[:, :], accum[:, w * 128 : (w + 1) * 128], own[:, :]
                    )
                    nc.vector.tensor_scalar(
                        t1[:, :], t1[:, :], dinv_sb[:, w : w + 1], None,
                        mybir.AluOpType.mult,
                    )
                    nc.vector.tensor_add(t1[:, :], t1[:, :], xw[:, :])
                    nc.vector.tensor_add(t1[:, :], t1[:, :], b1_sb[:, :])
                    h1w = ep.tile([128, D], F32, tag="h1w")
                    nc.scalar.activation(
                        h1w[:, :], t1[:, :], mybir.ActivationFunctionType.Relu
                    )
                    nc.sync.dma_start(out=h1_d[w, :, :], in_=h1w[:, :])
                    h1b = ep.tile([128, D], BF16, tag="h1b")
                    nc.vector.tensor_copy(h1b[:, :], h1w[:, :])
                    pt = p2.tile([128, D], BF16, tag="pt")
                    nc.tensor.transpose(pt[:, :], h1b[:, :], identb_sb[:, :])
                    h1T = ep.tile([128, D], BF16, tag="h1T")
                    nc.vector.tensor_copy(h1T[:, :], pt[:, :])
                    pg = p2.tile([128, D], F32, tag="pg")
                    nc.tensor.matmul(
                        pg[:, :], h1T[:, :], w2_sb[:, :], start=True, stop=True
                    )
                    g2t = ep.tile([128, D], BF16, tag="g2t")
                    nc.vector.tensor_scalar(
                        g2t[:, :], pg[:, :], dinv_sb[:, w : w + 1], None,
                        mybir.AluOpType.mult,
                    )
                    nc.sync.dma_start(
                        out=gq[1][q][wq * 128 : (wq + 1) * 128, :], in_=g2t[:, :]
                    )
                    if wq == p.WQ - 1 or w == p.real_w - 1:
                        if w == p.real_w - 1 and wq != p.WQ - 1:
                            # zero-fill remaining pad windows of this quarter
                            z = ep.tile([128, D], BF16, tag="z")
                            nc.vector.memset(z[:, :], 0.0)
                            for wq2 in range(wq + 1, p.WQ):
                                nc.sync.dma_start(
                                    out=gq[1][q][
                                        wq2 * 128 : (wq2 + 1) * 128, :
                                    ],
                                    in_=z[:, :],
                                )
                        nc.gpsimd.collective_compute(
                            "AllGather",
                            mybir.AluOpType.bypass,
                            replica_groups=RG,
                            ins=[gq[1][q][:, :]],
                            outs=[tables[1][q][:, :]],
                        )

        def epi2():
            with tc.tile_pool(name="e2", bufs=6) as ep:
                for w in range(p.real_w):
                    q, wq = w // p.WQ, w % p.WQ
                    own = ep.tile([128, D], BF16, tag="own")
                    nc.sync.dma_start(
                        out=own[:, :], in_=gq[1][q][wq * 128 : (wq + 1) * 128, :]
                    )
                    h1w = ep.tile([128, D], F32, tag="h1w")
                    nc.sync.dma_start(out=h1w[:, :], in_=h1_d[w, :, :])
                    t1 = ep.tile([128, D], F32, tag="t1")
                    nc.vector.tensor_add(
                        t1[:, :], accum[:, w * 128 : (w + 1) * 128], own[:, :]
                    )
                    nc.vector.tensor_scalar(
                        t1[:, :], t1[:, :], dinv_sb[:, w : w + 1], None,
                        mybir.AluOpType.mult,
                    )
                    nc.vector.tensor_add(t1[:, :], t1[:, :], h1w[:, :])
                    nc.vector.tensor_add(t1[:, :], t1[:, :], b2_sb[:, :])
                    nc.sync.dma_start(out=out_d[w, :, :], in_=t1[:, :])

        g1_phase()
        agg_phase(1)
        epi1()
        agg_phase(2)
        epi2()
        ctx.close()

    nc.compile()
    return nc


# --------------------------------------------------------------------------
# Entry point
# --------------------------------------------------------------------------
def kernel(x, edge_index, W1, b1, W2, b2):
    x = np.asarray(x)
    edge_index = np.asarray(edge_index)
    N = x.shape[0]
    p, in_maps = _host_prepare(
        np.asarray(x, dtype=np.float32),
        edge_index,
        np.asarray(W1, dtype=np.float32),
        np.asarray(b1, dtype=np.float32),
        np.asarray(W2, dtype=np.float32),
        np.asarray(b2, dtype=np.float32),
    )
    nc = _build_program(p)

    if os.environ.get("GCN_SIM"):
        from concourse import bass_interp

        sim = bass_interp.MultiCoreSim(nc, N_CORES)
        for c in range(N_CORES):
            for k, v in in_maps[c].items():
                sim.cores[c].tensor(k)[:] = v
        sim.simulate(check_with_hw=False)
        outs = [sim.cores[c].mem_tensor("out") for c in range(N_CORES)]
    else:
        from concourse.bass_utils import run_bass_kernel_spmd

        res = run_bass_kernel_spmd(
            nc,
            in_maps,
            list(range(N_CORES)),
            trace=bool(os.environ.get("GCN_TRACE")),
        )
        kernel.last_result = res
        outs = [res.results[c]["out"] for c in range(N_CORES)]

    full = np.concatenate(
        [
            np.asarray(o, dtype=np.float32).reshape(p.SH, D)[
                : min(p.shard, N - c * p.shard)
            ]
            for c, o in enumerate(outs)
        ],
        axis=0,
    )
    return full.astype(np.float32)


# revision 3
# speedup vs baseline: 1.6854x; 1.2287x over previous
"""Two-layer residual GCN (PyG GCNConv-style) on 8 Trainium2 NeuronCores, v2.

Same dst-sharded skeleton as v1, tuned around the real bottleneck (GpSimd
descriptor generation for dma_gather, ~8 ns per gathered row, serial):

  - Self-loops are not materialized as edges: the epilogue adds the own-window
    table row (g_raw[d]*dinv[d]) before the final dinv[d] scale, giving
    dinv[d]^2*g_raw[d] exactly. Saves ~3% of gather descriptors.
  - Gather slots are sized per (chunk, window) cell to the max edge count over
    the 8 cores (SPMD program is shared), not rounded up to 128 per cell:
    saves ~5% descriptors. Blocks that straddle a window boundary get one
    masked one-hot pass per extra window.
  - dma_gather instructions rotate across 4 SWDGE queues: desc-gen for the
    next gather is not blocked behind the previous gather's ring drain.
  - Gathers are issued chunk-major so no gather waits on a later chunk's
    AllGather; per-window sums accumulate in an SBUF fp32 tile.
  - Tables, gathered messages, one-hots, and matmuls are bf16 (halves gather
    DMA bytes and AllGather wire, 2x LDWEIGHTS); the residual path (x, h1,
    accumulators, output) stays fp32.
  - dinv = 1/sqrt(deg+1) comes from the host.
"""

import os
import sys

import numpy as np

for _p in ("/opt/trn_rl_repo",):
    if _p not in sys.path and os.path.isdir(_p):
        sys.path.insert(0, _p)

from concourse import bacc, bass, mybir
from concourse.tile import TileContext

F32 = mybir.dt.float32
BF16 = mybir.dt.bfloat16
I16 = mybir.dt.int16

N_CORES = 8
N_CHUNKS = 4
GROUP_W = 4  # windows per gather unit
D = 128
QROT = int(os.environ.get("GCN_QROT", "4"))
SCRATCH = int(os.environ.get("GCN_SCRATCH", "49152"))


# --------------------------------------------------------------------------
# Planning (host): common SPMD structure + per-core data
# --------------------------------------------------------------------------
class Plan:
    def __init__(self, n_nodes, n_edges):
        self.N = n_nodes
        self.E = n_edges
        self.shard = -(-n_nodes // N_CORES)
        self.qrows = -(-(-(-self.shard // N_CHUNKS)) // 128) * 128
        self.SH = N_CHUNKS * self.qrows
        self.W = self.SH // 128
        self.WQ = self.qrows // 128
        self.TROWS = N_CORES * self.qrows
        assert self.TROWS <= 32767
        self.real_w = -(-self.shard // 128)


def _host_prepare(x, edge_index, W1, b1, W2, b2):
    import ml_dtypes

    N, d = x.shape
    assert d == D
    E = edge_index.shape[1]
    p = Plan(N, E)

    src = np.ascontiguousarray(edge_index[0]).astype(np.int64)
    dst = np.ascontiguousarray(edge_index[1]).astype(np.int64)

    core = dst // p.shard
    l_dst = dst - core * p.shard
    win = l_dst // 128
    dr = (l_dst % 128).astype(np.int64)
    r_src = src // p.shard
    l_src = src - r_src * p.shard
    q_src = l_src // p.qrows
    t_row = r_src * p.qrows + (l_src - q_src * p.qrows)

    # cell = (chunk, window); common size = max over cores
    ncell = N_CHUNKS * p.W
    cellid = q_src * p.W + win
    counts = np.zeros((N_CORES, ncell), dtype=np.int64)
    for c in range(N_CORES):
        counts[c] = np.bincount(cellid[core == c], minlength=ncell)
    L = counts.max(axis=0).reshape(N_CHUNKS, p.W)  # [chunk, window]

    # common slot layout: units = (chunk q, group g of GROUP_W windows)
    n_groups = -(-p.W // GROUP_W)
    units = []  # (q, icol0, n_slot, nblk, passes, windows)
    # passes: list of (blk, w, pcol, start, stop)
    wv_all = {}  # unit -> per-slot window id (-1 pad)
    cell_off = np.zeros((N_CHUNKS, p.W), dtype=np.int64)  # slot offset of cell
    icol = 0
    pcol = 0
    for q in range(N_CHUNKS):
        for g in range(n_groups):
            ws = list(range(g * GROUP_W, min((g + 1) * GROUP_W, p.W)))
            lens = [int(L[q, w]) for w in ws]
            tot = int(sum(lens))
            if tot == 0:
                units.append(None)
                continue
            n_slot = -(-tot // 128) * 128
            wv = np.full(n_slot, -1, dtype=np.int64)
            off = 0
            for w, ln in zip(ws, lens):
                cell_off[q, w] = icol * 16 + off
                wv[off : off + ln] = w
                off += ln
            nblk = -(-n_slot // 128)
            tmp = []
            for b in range(nblk):
                bw = wv[b * 128 : (b + 1) * 128]
                for w in np.unique(bw[bw >= 0]):
                    tmp.append((b, int(w), pcol))
                    pcol += 1
            first, last = {}, {}
            for b, w, pc in tmp:
                if w not in first:
                    first[w] = pc
                last[w] = pc
            passes = [
                (b, w, pc, pc == first[w], pc == last[w]) for b, w, pc in tmp
            ]
            units.append((q, icol, n_slot, nblk, passes, ws))
            wv_all[(q, g)] = wv
            icol += n_slot // 16
    p.units = units
    p.n_groups = n_groups
    p.TOTC = max(icol, 1)
    p.PASST = max(pcol, 1)
    p.MAXBLK = max((u[3] for u in units if u), default=1)
    p.MAXPASS = max((len(u[4]) for u in units if u), default=1)

    # degree incl self-loop -> dinv
    deg = np.bincount(dst, minlength=N).astype(np.float64) + 1.0
    dinv_full = (1.0 / np.sqrt(deg)).astype(np.float32)

    iota = np.tile(np.arange(128, dtype=np.float32), (128, 1)).astype(
        ml_dtypes.bfloat16
    )
    ident_bf = np.eye(128, dtype=np.float32).astype(ml_dtypes.bfloat16)
    b1t = np.tile(b1.astype(np.float32), (128, 1))
    b2t = np.tile(b2.astype(np.float32), (128, 1))

    in_maps = []
    for c in range(N_CORES):
        m = core == c
        ec, ew, et, edr = cellid[m], win[m], t_row[m], dr[m]
        order = np.argsort(ec, kind="stable")
        ec, ew, et, edr = ec[order], ew[order], et[order], edr[order]
        # slot position: cell offset + rank within cell
        cstart = np.zeros(ncell + 1, dtype=np.int64)
        np.cumsum(np.bincount(ec, minlength=ncell), out=cstart[1:])
        rank = np.arange(len(ec)) - cstart[ec]
        qq = ec // p.W
        wwin = ec % p.W
        slot = cell_off[qq, wwin] + rank

        idx_flat = np.zeros(p.TOTC * 16, dtype=np.int16)
        drel_slot = np.full(p.TOTC * 16, -1.0, dtype=np.float32)
        idx_flat[slot] = et.astype(np.int16)
        drel_slot[slot] = edr.astype(np.float32)

        # idx wrapped [16, TOTC] then replicated x8
        idx16 = idx_flat.reshape(p.TOTC, 16).T
        idx128 = np.tile(idx16, (8, 1))

        # per-pass drel columns [128, PASST]: block slots masked to the pass's
        # window via the common window layout
        drel = np.full((128, p.PASST), -1.0, dtype=np.float32)
        for u in units:
            if u is None:
                continue
            q, ic0, n_slot, nblk, passes, ws = u
            g = (ws[0]) // GROUP_W
            wv = wv_all[(q, g)]
            base = ic0 * 16
            for b, w, pc, st, sp in passes:
                s0 = b * 128
                s1 = min((b + 1) * 128, n_slot)
                seg_w = wv[s0:s1]
                seg_d = drel_slot[base + s0 : base + s1]
                col = np.where(seg_w == w, seg_d, -1.0).astype(np.float32)
                full = np.full(128, -1.0, dtype=np.float32)
                full[: len(col)] = col
                drel[:, pc] = full

        n0 = c * p.shard
        nreal = max(0, min(N - n0, p.shard))
        x_pad = np.zeros((p.SH, D), dtype=np.float32)
        x_pad[:nreal] = x[n0 : n0 + nreal]
        x_tiled = np.ascontiguousarray(x_pad.reshape(p.W, 128, D))
        xT = np.ascontiguousarray(x_pad.T).astype(ml_dtypes.bfloat16)

        dinv_pad = np.ones(p.SH, dtype=np.float32)
        dinv_pad[:nreal] = dinv_full[n0 : n0 + nreal]
        dinv_t = np.ascontiguousarray(dinv_pad.reshape(p.W, 128).T)

        in_maps.append(
            {
                "x_tiled": x_tiled,
                "xT": xT,
                "W1": W1.astype(ml_dtypes.bfloat16),
                "W2": W2.astype(ml_dtypes.bfloat16),
                "b1t": b1t,
                "b2t": b2t,
                "iota": iota,
                "ident_bf": ident_bf,
                "dinv": dinv_t,
                "idx16": idx128,
                "dstrel": drel.astype(ml_dtypes.bfloat16),
            }
        )
    return p, in_maps


# --------------------------------------------------------------------------
# Device program
# --------------------------------------------------------------------------
def _build_program(p: Plan):
    from contextlib import ExitStack

    nc = bacc.Bacc(
        "TRN2",
        target_bir_lowering=False,
        debug=False,
        num_devices=N_CORES,
        num_swdge_queues=max(QROT, 1),
        dynamic_dma_scratch_size=SCRATCH,
    )
    RG = [list(range(N_CORES))]

    x_tiled = nc.dram_tensor("x_tiled", [p.W, 128, D], F32, kind="ExternalInput")
    xT_d = nc.dram_tensor("xT", [D, p.SH], BF16, kind="ExternalInput")
    W1_d = nc.dram_tensor("W1", [D, D], BF16, kind="ExternalInput")
    W2_d = nc.dram_tensor("W2", [D, D], BF16, kind="ExternalInput")
    b1_d = nc.dram_tensor("b1t", [128, D], F32, kind="ExternalInput")
    b2_d = nc.dram_tensor("b2t", [128, D], F32, kind="ExternalInput")
    iota_d = nc.dram_tensor("iota", [128, 128], BF16, kind="ExternalInput")
    identb_d = nc.dram_tensor("ident_bf", [128, 128], BF16, kind="ExternalInput")
    dinv_d = nc.dram_tensor("dinv", [128, p.W], F32, kind="ExternalInput")
    idx_d = nc.dram_tensor("idx16", [128, p.TOTC], I16, kind="ExternalInput")
    drel_d = nc.dram_tensor("dstrel", [128, p.PASST], BF16, kind="ExternalInput")

    out_d = nc.dram_tensor("out", [p.W, 128, D], F32, kind="ExternalOutput")

    gq = [
        [nc.dram_tensor(f"g{l}q{q}", [p.qrows, D], BF16) for q in range(N_CHUNKS)]
        for l in (1, 2)
    ]
    tables = [
        [
            nc.dram_tensor(f"t{l}q{q}", [p.TROWS, D], BF16, addr_space="Shared")
            for q in range(N_CHUNKS)
        ]
        for l in (1, 2)
    ]
    h1_d = nc.dram_tensor("h1", [p.W, 128, D], F32)

    gq_i = 0  # rotating swdge queue counter

    with TileContext(nc) as tc:
        ctx = ExitStack()
        cst = ctx.enter_context(tc.tile_pool(name="cst", bufs=1))
        w1_sb = cst.tile([D, D], BF16, tag="w1")
        w2_sb = cst.tile([D, D], BF16, tag="w2")
        b1_sb = cst.tile([128, D], F32, tag="b1")
        b2_sb = cst.tile([128, D], F32, tag="b2")
        iota_sb = cst.tile([128, 128], BF16, tag="iota")
        identb_sb = cst.tile([128, 128], BF16, tag="identb")
        dinv_sb = cst.tile([128, p.W], F32, tag="dinv")
        for t, dr_ in (
            (w1_sb, W1_d),
            (w2_sb, W2_d),
            (b1_sb, b1_d),
            (b2_sb, b2_d),
            (iota_sb, iota_d),
            (identb_sb, identb_d),
            (dinv_sb, dinv_d),
        ):
            nc.sync.dma_start(out=t[:, :], in_=dr_[:, :])

        accum_pool = ctx.enter_context(tc.tile_pool(name="acc", bufs=1))
        accum = accum_pool.tile([128, p.W * 128], F32, tag="accum")

        def g1_phase():
            with tc.tile_pool(name="xT", bufs=1) as xp, \
                 tc.tile_pool(name="g1o", bufs=4) as go, \
                 tc.tile_pool(name="g1p", bufs=2, space="PSUM") as gp:
                xT_sb = xp.tile([D, p.SH], BF16, tag="xT")
                nc.sync.dma_start(out=xT_sb[:, :], in_=xT_d[:, :])
                for q in range(N_CHUNKS):
                    for wq in range(p.WQ):
                        w = q * p.WQ + wq
                        ps = gp.tile([128, D], F32, tag="ps")
                        nc.tensor.matmul(
                            ps[:, :],
                            xT_sb[:, w * 128 : (w + 1) * 128],
                            w1_sb[:, :],
                            start=True,
                            stop=True,
                        )
                        gt = go.tile([128, D], BF16, tag="gt")
                        nc.vector.tensor_scalar(
                            gt[:, :], ps[:, :], dinv_sb[:, w : w + 1], None,
                            mybir.AluOpType.mult,
                        )
                        nc.sync.dma_start(
                            out=gq[0][q][wq * 128 : (wq + 1) * 128, :],
                            in_=gt[:, :],
                        )
                    nc.gpsimd.collective_compute(
                        "AllGather",
                        mybir.AluOpType.bypass,
                        replica_groups=RG,
                        ins=[gq[0][q][:, :]],
                        outs=[tables[0][q][:, :]],
                    )

        def agg_phase(layer, epi_cb=None):
            nonlocal gq_i
            table = tables[layer - 1]
            nc.vector.memset(accum[:, :], 0.0)
            with (
                tc.tile_pool(name=f"mt{layer}", bufs=3) as mp,
                tc.tile_pool(name=f"at{layer}", bufs=3) as ap_,
                tc.tile_pool(name=f"ix{layer}", bufs=3) as ip,
                tc.tile_pool(name=f"dl{layer}", bufs=3) as dp,
                tc.tile_pool(name=f"ps{layer}", bufs=4, space="PSUM") as pp,
            ):
                emitted = set()
                for u in p.units:
                    if u is None:
                        continue
                    q, ic0, n_slot, nblk, passes, ws = u
                    it = ip.tile([128, n_slot // 16], I16, tag="it")
                    nc.sync.dma_start(
                        out=it[:, :], in_=idx_d[:, ic0 : ic0 + n_slot // 16]
                    )
                    npass = len(passes)
                    dt_ = dp.tile([128, npass], BF16, tag="dl")
                    p0 = passes[0][2]
                    nc.sync.dma_start(
                        out=dt_[:, :], in_=drel_d[:, p0 : p0 + npass]
                    )
                    mt = mp.tile([128, p.MAXBLK, 128], BF16, tag="mt")
                    nc.gpsimd.dma_gather(
                        out_ap=mt[:, :nblk, :],
                        in_ap=table[q][:, :],
                        idxs_ap=it[:, :],
                        num_idxs=n_slot,
                        num_idxs_reg=n_slot,
                        elem_size=D,
                        single_packet=False,
                        queue_num=gq_i % max(QROT, 1),
                    )
                    gq_i += 1
                    at = ap_.tile([128, npass, 128], BF16, tag="at")
                    nc.vector.tensor_tensor(
                        at[:, :, :],
                        iota_sb.unsqueeze(1).broadcast_to([128, npass, 128]),
                        dt_.unsqueeze(2).broadcast_to([128, npass, 128]),
                        mybir.AluOpType.is_equal,
                    )
                    psums = {}
                    for b, w, pc, st, sp in passes:
                        if w not in psums:
                            psums[w] = pp.tile(
                                [128, D], F32, tag="ps", name=f"ps{layer}_{q}_{w}"
                            )
                        nc.tensor.matmul(
                            psums[w][:, :],
                            at[:, pc - p0, :],
                            mt[:, b, :],
                            start=st,
                            stop=sp,
                        )
                    for w in sorted(psums):
                        nc.vector.tensor_add(
                            accum[:, w * 128 : (w + 1) * 128],
                            accum[:, w * 128 : (w + 1) * 128],
                            psums[w][:, :],
                        )
                    # once the last chunk's partial sums for this window group
                    # are in, its epilogue can run under the remaining gathers
                    if epi_cb is not None and q == N_CHUNKS - 1:
                        for w in ws:
                            if w < p.real_w and w not in emitted:
                                epi_cb(w)
                                emitted.add(w)
                if epi_cb is not None:
                    for w in range(p.real_w):
                        if w not in emitted:
                            epi_cb(w)

        def emit_epi1(w, ep, p2):
            q, wq = w // p.WQ, w % p.WQ
            own = ep.tile([128, D], BF16, tag="own")
            nc.sync.dma_start(
                out=own[:, :], in_=gq[0][q][wq * 128 : (wq + 1) * 128, :]
            )
            xw = ep.tile([128, D], F32, tag="xw")
            nc.sync.dma_start(out=xw[:, :], in_=x_tiled[w, :, :])
            t1 = ep.tile([128, D], F32, tag="t1")
            nc.vector.tensor_add(
                t1[:, :], accum[:, w * 128 : (w + 1) * 128], own[:, :]
            )
            nc.vector.tensor_scalar(
                t1[:, :], t1[:, :], dinv_sb[:, w : w + 1], None,
                mybir.AluOpType.mult,
            )
            nc.vector.tensor_add(t1[:, :], t1[:, :], xw[:, :])
            nc.vector.tensor_add(t1[:, :], t1[:, :], b1_sb[:, :])
            h1w = ep.tile([128, D], F32, tag="h1w")
            nc.scalar.activation(
                h1w[:, :], t1[:, :], mybir.ActivationFunctionType.Relu
            )
            nc.sync.dma_start(out=h1_d[w, :, :], in_=h1w[:, :])
            h1b = ep.tile([128, D], BF16, tag="h1b")
            nc.vector.tensor_copy(h1b[:, :], h1w[:, :])
            pt = p2.tile([128, D], BF16, tag="pt")
            nc.tensor.transpose(pt[:, :], h1b[:, :], identb_sb[:, :])
            h1T = ep.tile([128, D], BF16, tag="h1T")
            nc.vector.tensor_copy(h1T[:, :], pt[:, :])
            pg = p2.tile([128, D], F32, tag="pg")
            nc.tensor.matmul(
                pg[:, :], h1T[:, :], w2_sb[:, :], start=True, stop=True
            )
            g2t = ep.tile([128, D], BF16, tag="g2t")
            nc.vector.tensor_scalar(
                g2t[:, :], pg[:, :], dinv_sb[:, w : w + 1], None,
                mybir.AluOpType.mult,
            )
            nc.sync.dma_start(
                out=gq[1][q][wq * 128 : (wq + 1) * 128, :], in_=g2t[:, :]
            )
            if wq == p.WQ - 1 or w == p.real_w - 1:
                if w == p.real_w - 1 and wq != p.WQ - 1:
                    z = ep.tile([128, D], BF16, tag="z")
                    nc.vector.memset(z[:, :], 0.0)
                    for wq2 in range(wq + 1, p.WQ):
                        nc.sync.dma_start(
                            out=gq[1][q][wq2 * 128 : (wq2 + 1) * 128, :],
                            in_=z[:, :],
                        )
                nc.gpsimd.collective_compute(
                    "AllGather",
                    mybir.AluOpType.bypass,
                    replica_groups=RG,
                    ins=[gq[1][q][:, :]],
                    outs=[tables[1][q][:, :]],
                )

        def emit_epi2(w, ep):
            q, wq = w // p.WQ, w % p.WQ
            own = ep.tile([128, D], BF16, tag="own")
            nc.sync.dma_start(
                out=own[:, :], in_=gq[1][q][wq * 128 : (wq + 1) * 128, :]
            )
            h1w = ep.tile([128, D], F32, tag="h1w")
            nc.sync.dma_start(out=h1w[:, :], in_=h1_d[w, :, :])
            t1 = ep.tile([128, D], F32, tag="t1")
            nc.vector.tensor_add(
                t1[:, :], accum[:, w * 128 : (w + 1) * 128], own[:, :]
            )
            nc.vector.tensor_scalar(
                t1[:, :], t1[:, :], dinv_sb[:, w : w + 1], None,
                mybir.AluOpType.mult,
            )
            nc.vector.tensor_add(t1[:, :], t1[:, :], h1w[:, :])
            nc.vector.tensor_add(t1[:, :], t1[:, :], b2_sb[:, :])
            nc.sync.dma_start(out=out_d[w, :, :], in_=t1[:, :])

        g1_phase()
        with (
            tc.tile_pool(name="e1", bufs=6) as ep1,
            tc.tile_pool(name="e1p", bufs=2, space="PSUM") as p21,
        ):
            agg_phase(1, epi_cb=lambda w: emit_epi1(w, ep1, p21))
        with tc.tile_pool(name="e2", bufs=6) as ep2:
            agg_phase(2, epi_cb=lambda w: emit_epi2(w, ep2))
        ctx.close()

    nc.compile()
    return nc


# --------------------------------------------------------------------------
# Entry point
# --------------------------------------------------------------------------
def kernel(x, edge_index, W1, b1, W2, b2):
    x = np.asarray(x)
    edge_index = np.asarray(edge_index)
    N = x.shape[0]
    p, in_maps = _host_prepare(
        np.asarray(x, dtype=np.float32),
        edge_index,
        np.asarray(W1, dtype=np.float32),
        np.asarray(b1, dtype=np.float32),
        np.asarray(W2, dtype=np.float32),
        np.asarray(b2, dtype=np.float32),
    )
    nc = _build_program(p)

    if os.environ.get("GCN_SIM"):
        from concourse import bass_interp

        sim = bass_interp.MultiCoreSim(nc, N_CORES)
        for c in range(N_CORES):
            for k, v in in_maps[c].items():
                sim.cores[c].tensor(k)[:] = v
        sim.simulate(check_with_hw=False)
        outs = [sim.cores[c].mem_tensor("out") for c in range(N_CORES)]
    else:
        from concourse.bass_utils import run_bass_kernel_spmd

        res = run_bass_kernel_spmd(
            nc,
            in_maps,
            list(range(N_CORES)),
            trace=bool(os.environ.get("GCN_TRACE")),
        )
        kernel.last_result = res
        outs = [res.results[c]["out"] for c in range(N_CORES)]

    full = np.concatenate(
        [
            np.asarray(o, dtype=np.float32).reshape(p.SH, D)[
                : min(p.shard, N - c * p.shard)
            ]
            for c, o in enumerate(outs)
        ],
        axis=0,
    )
    return full.astype(np.float32)


# revision 5
# speedup vs baseline: 1.7134x; 1.0167x over previous
"""Two-layer residual GCN (PyG GCNConv-style) on 8 Trainium2 NeuronCores, v2.

Same dst-sharded skeleton as v1, tuned around the real bottleneck (GpSimd
descriptor generation for dma_gather, ~8 ns per gathered row, serial):

  - Self-loops are not materialized as edges: the epilogue adds the own-window
    table row (g_raw[d]*dinv[d]) before the final dinv[d] scale, giving
    dinv[d]^2*g_raw[d] exactly. Saves ~3% of gather descriptors.
  - Gather slots are sized per (chunk, window) cell to the max edge count over
    the 8 cores (SPMD program is shared), not rounded up to 128 per cell:
    saves ~5% descriptors. Blocks that straddle a window boundary get one
    masked one-hot pass per extra window.
  - dma_gather instructions rotate across 4 SWDGE queues: desc-gen for the
    next gather is not blocked behind the previous gather's ring drain.
  - Gathers are issued chunk-major so no gather waits on a later chunk's
    AllGather; per-window sums accumulate in an SBUF fp32 tile.
  - Tables, gathered messages, one-hots, and matmuls are bf16 (halves gather
    DMA bytes and AllGather wire, 2x LDWEIGHTS); the residual path (x, h1,
    accumulators, output) stays fp32.
  - dinv = 1/sqrt(deg+1) comes from the host.
"""

import os
import sys

import numpy as np

for _p in ("/opt/trn_rl_repo",):
    if _p not in sys.path and os.path.isdir(_p):
        sys.path.insert(0, _p)

from concourse import bacc, bass, mybir
from concourse.tile import TileContext

F32 = mybir.dt.float32
BF16 = mybir.dt.bfloat16
I16 = mybir.dt.int16

N_CORES = 8
N_CHUNKS = 4
GROUP_W = 4  # windows per gather unit
D = 128
QROT = int(os.environ.get("GCN_QROT", "4"))
SCRATCH = int(os.environ.get("GCN_SCRATCH", "49152"))


# --------------------------------------------------------------------------
# Planning (host): common SPMD structure + per-core data
# --------------------------------------------------------------------------
class Plan:
    def __init__(self, n_nodes, n_edges):
        self.N = n_nodes
        self.E = n_edges
        self.shard = -(-n_nodes // N_CORES)
        self.qrows = -(-(-(-self.shard // N_CHUNKS)) // 128) * 128
        self.SH = N_CHUNKS * self.qrows
        self.W = self.SH // 128
        self.WQ = self.qrows // 128
        self.TROWS = N_CORES * self.qrows
        assert self.TROWS <= 32767
        self.real_w = -(-self.shard // 128)


def _host_prepare(x, edge_index, W1, b1, W2, b2):
    import ml_dtypes

    N, d = x.shape
    assert d == D
    E = edge_index.shape[1]
    p = Plan(N, E)

    src = np.ascontiguousarray(edge_index[0]).astype(np.int64)
    dst = np.ascontiguousarray(edge_index[1]).astype(np.int64)

    core = dst // p.shard
    l_dst = dst - core * p.shard
    win = l_dst // 128
    dr = (l_dst % 128).astype(np.int64)
    r_src = src // p.shard
    l_src = src - r_src * p.shard
    q_src = l_src // p.qrows
    t_row = r_src * p.qrows + (l_src - q_src * p.qrows)

    # cell = (chunk, window); common size = max over cores
    ncell = N_CHUNKS * p.W
    cellid = q_src * p.W + win
    counts = np.zeros((N_CORES, ncell), dtype=np.int64)
    for c in range(N_CORES):
        counts[c] = np.bincount(cellid[core == c], minlength=ncell)
    L = counts.max(axis=0).reshape(N_CHUNKS, p.W)  # [chunk, window]

    # common slot layout: units = (chunk q, group g of GROUP_W windows)
    n_groups = -(-p.W // GROUP_W)
    units = []  # (q, icol0, n_slot, nblk, passes, windows)
    # passes: list of (blk, w, pcol, start, stop)
    wv_all = {}  # unit -> per-slot window id (-1 pad)
    cell_off = np.zeros((N_CHUNKS, p.W), dtype=np.int64)  # slot offset of cell
    icol = 0
    pcol = 0
    for q in range(N_CHUNKS):
        for g in range(n_groups):
            ws = list(range(g * GROUP_W, min((g + 1) * GROUP_W, p.W)))
            lens = [int(L[q, w]) for w in ws]
            tot = int(sum(lens))
            if tot == 0:
                units.append(None)
                continue
            n_slot = -(-tot // 128) * 128
            wv = np.full(n_slot, -1, dtype=np.int64)
            off = 0
            for w, ln in zip(ws, lens):
                cell_off[q, w] = icol * 16 + off
                wv[off : off + ln] = w
                off += ln
            nblk = -(-n_slot // 128)
            tmp = []
            for b in range(nblk):
                bw = wv[b * 128 : (b + 1) * 128]
                for w in np.unique(bw[bw >= 0]):
                    tmp.append((b, int(w), pcol))
                    pcol += 1
            first, last = {}, {}
            for b, w, pc in tmp:
                if w not in first:
                    first[w] = pc
                last[w] = pc
            passes = [
                (b, w, pc, pc == first[w], pc == last[w]) for b, w, pc in tmp
            ]
            units.append((q, icol, n_slot, nblk, passes, ws))
            wv_all[(q, g)] = wv
            icol += n_slot // 16
    p.units = units
    p.n_groups = n_groups
    p.TOTC = max(icol, 1)
    p.PASST = max(pcol, 1)
    p.MAXBLK = max((u[3] for u in units if u), default=1)
    p.MAXPASS = max((len(u[4]) for u in units if u), default=1)

    # degree incl self-loop -> dinv
    deg = np.bincount(dst, minlength=N).astype(np.float64) + 1.0
    dinv_full = (1.0 / np.sqrt(deg)).astype(np.float32)

    iota = np.tile(np.arange(128, dtype=np.float32), (128, 1)).astype(
        ml_dtypes.bfloat16
    )
    ident_bf = np.eye(128, dtype=np.float32).astype(ml_dtypes.bfloat16)
    b1t = np.tile(b1.astype(np.float32), (128, 1))
    b2t = np.tile(b2.astype(np.float32), (128, 1))

    in_maps = []
    for c in range(N_CORES):
        m = core == c
        ec, ew, et, edr = cellid[m], win[m], t_row[m], dr[m]
        order = np.argsort(ec, kind="stable")
        ec, ew, et, edr = ec[order], ew[order], et[order], edr[order]
        # slot position: cell offset + rank within cell
        cstart = np.zeros(ncell + 1, dtype=np.int64)
        np.cumsum(np.bincount(ec, minlength=ncell), out=cstart[1:])
        rank = np.arange(len(ec)) - cstart[ec]
        qq = ec // p.W
        wwin = ec % p.W
        slot = cell_off[qq, wwin] + rank

        idx_flat = np.zeros(p.TOTC * 16, dtype=np.int16)
        drel_slot = np.full(p.TOTC * 16, -1.0, dtype=np.float32)
        idx_flat[slot] = et.astype(np.int16)
        drel_slot[slot] = edr.astype(np.float32)

        # idx wrapped [16, TOTC] then replicated x8
        idx16 = idx_flat.reshape(p.TOTC, 16).T
        idx128 = np.tile(idx16, (8, 1))

        # per-pass drel columns [128, PASST]: block slots masked to the pass's
        # window via the common window layout
        drel = np.full((128, p.PASST), -1.0, dtype=np.float32)
        for u in units:
            if u is None:
                continue
            q, ic0, n_slot, nblk, passes, ws = u
            g = (ws[0]) // GROUP_W
            wv = wv_all[(q, g)]
            base = ic0 * 16
            for b, w, pc, st, sp in passes:
                s0 = b * 128
                s1 = min((b + 1) * 128, n_slot)
                seg_w = wv[s0:s1]
                seg_d = drel_slot[base + s0 : base + s1]
                col = np.where(seg_w == w, seg_d, -1.0).astype(np.float32)
                full = np.full(128, -1.0, dtype=np.float32)
                full[: len(col)] = col
                drel[:, pc] = full

        n0 = c * p.shard
        nreal = max(0, min(N - n0, p.shard))
        x_pad = np.zeros((p.SH, D), dtype=np.float32)
        x_pad[:nreal] = x[n0 : n0 + nreal]
        x_tiled = np.ascontiguousarray(x_pad.reshape(p.W, 128, D))
        xT = np.ascontiguousarray(x_pad.T).astype(ml_dtypes.bfloat16)

        dinv_pad = np.ones(p.SH, dtype=np.float32)
        dinv_pad[:nreal] = dinv_full[n0 : n0 + nreal]
        dinv_t = np.ascontiguousarray(dinv_pad.reshape(p.W, 128).T)

        in_maps.append(
            {
                "x_tiled": x_tiled,
                "xT": xT,
                "W1": W1.astype(ml_dtypes.bfloat16),
                "W2": W2.astype(ml_dtypes.bfloat16),
                "b1t": b1t,
                "b2t": b2t,
                "iota": iota,
                "ident_bf": ident_bf,
                "dinv": dinv_t,
                "idx16": idx128,
                "dstrel": drel.astype(ml_dtypes.bfloat16),
            }
        )
    return p, in_maps


# --------------------------------------------------------------------------
# Device program
# --------------------------------------------------------------------------
def _build_program(p: Plan):
    from contextlib import ExitStack

    nc = bacc.Bacc(
        "TRN2",
        target_bir_lowering=False,
        debug=False,
        num_devices=N_CORES,
        num_swdge_queues=max(QROT, 1),
        dynamic_dma_scratch_size=SCRATCH,
    )
    RG = [list(range(N_CORES))]

    x_tiled = nc.dram_tensor("x_tiled", [p.W, 128, D], F32, kind="ExternalInput")
    xT_d = nc.dram_tensor("xT", [D, p.SH], BF16, kind="ExternalInput")
    W1_d = nc.dram_tensor("W1", [D, D], BF16, kind="ExternalInput")
    W2_d = nc.dram_tensor("W2", [D, D], BF16, kind="ExternalInput")
    b1_d = nc.dram_tensor("b1t", [128, D], F32, kind="ExternalInput")
    b2_d = nc.dram_tensor("b2t", [128, D], F32, kind="ExternalInput")
    iota_d = nc.dram_tensor("iota", [128, 128], BF16, kind="ExternalInput")
    identb_d = nc.dram_tensor("ident_bf", [128, 128], BF16, kind="ExternalInput")
    dinv_d = nc.dram_tensor("dinv", [128, p.W], F32, kind="ExternalInput")
    idx_d = nc.dram_tensor("idx16", [128, p.TOTC], I16, kind="ExternalInput")
    drel_d = nc.dram_tensor("dstrel", [128, p.PASST], BF16, kind="ExternalInput")

    out_d = nc.dram_tensor("out", [p.W, 128, D], F32, kind="ExternalOutput")

    gq = [
        [nc.dram_tensor(f"g{l}q{q}", [p.qrows, D], BF16) for q in range(N_CHUNKS)]
        for l in (1, 2)
    ]
    tables = [
        [
            nc.dram_tensor(f"t{l}q{q}", [p.TROWS, D], BF16, addr_space="Shared")
            for q in range(N_CHUNKS)
        ]
        for l in (1, 2)
    ]
    h1_d = nc.dram_tensor("h1", [p.W, 128, D], F32)

    gq_i = 0  # rotating swdge queue counter

    with TileContext(nc) as tc:
        ctx = ExitStack()
        cst = ctx.enter_context(tc.tile_pool(name="cst", bufs=1))
        w1_sb = cst.tile([D, D], BF16, tag="w1")
        w2_sb = cst.tile([D, D], BF16, tag="w2")
        b1_sb = cst.tile([128, D], F32, tag="b1")
        b2_sb = cst.tile([128, D], F32, tag="b2")
        iota_sb = cst.tile([128, 128], BF16, tag="iota")
        identb_sb = cst.tile([128, 128], BF16, tag="identb")
        dinv_sb = cst.tile([128, p.W], F32, tag="dinv")
        for t, dr_ in (
            (w1_sb, W1_d),
            (w2_sb, W2_d),
            (b1_sb, b1_d),
            (b2_sb, b2_d),
            (iota_sb, iota_d),
            (identb_sb, identb_d),
            (dinv_sb, dinv_d),
        ):
            nc.sync.dma_start(out=t[:, :], in_=dr_[:, :])

        accum_pool = ctx.enter_context(tc.tile_pool(name="acc", bufs=1))
        accum = accum_pool.tile([128, p.W * 128], F32, tag="accum")

        def g1_phase():
            with tc.tile_pool(name="xT", bufs=1) as xp, \
                 tc.tile_pool(name="g1o", bufs=4) as go, \
                 tc.tile_pool(name="g1p", bufs=2, space="PSUM") as gp:
                xT_sb = xp.tile([D, p.SH], BF16, tag="xT")
                nc.sync.dma_start(out=xT_sb[:, :], in_=xT_d[:, :])
                for q in range(N_CHUNKS):
                    for wq in range(p.WQ):
                        w = q * p.WQ + wq
                        ps = gp.tile([128, D], F32, tag="ps")
                        nc.tensor.matmul(
                            ps[:, :],
                            xT_sb[:, w * 128 : (w + 1) * 128],
                            w1_sb[:, :],
                            start=True,
                            stop=True,
                        )
                        gt = go.tile([128, D], BF16, tag="gt")
                        nc.vector.tensor_scalar(
                            gt[:, :], ps[:, :], dinv_sb[:, w : w + 1], None,
                            mybir.AluOpType.mult,
                        )
                        nc.sync.dma_start(
                            out=gq[0][q][wq * 128 : (wq + 1) * 128, :],
                            in_=gt[:, :],
                        )
                    nc.gpsimd.collective_compute(
                        "AllGather",
                        mybir.AluOpType.bypass,
                        replica_groups=RG,
                        ins=[gq[0][q][:, :]],
                        outs=[tables[0][q][:, :]],
                    )

        epi1_ag_queue = []

        def agg_phase(layer, epi_cb=None):
            nonlocal gq_i
            table = tables[layer - 1]
            nc.vector.memset(accum[:, :], 0.0)
            pending_ags = epi1_ag_queue  # epi_cb appends here
            ag_ready = []
            with (
                tc.tile_pool(name=f"mt{layer}", bufs=3) as mp,
                tc.tile_pool(name=f"at{layer}", bufs=3) as ap_,
                tc.tile_pool(name=f"ix{layer}", bufs=3) as ip,
                tc.tile_pool(name=f"dl{layer}", bufs=3) as dp,
                tc.tile_pool(name=f"ps{layer}", bufs=4, space="PSUM") as pp,
            ):
                emitted = set()
                for u in p.units:
                    if u is None:
                        continue
                    # fire AGs queued >=2 units ago: their input DMAs have
                    # drained, so the trigger won't stall the gather stream
                    for ag in ag_ready:
                        ag()
                    del ag_ready[:]
                    ag_ready.extend(pending_ags)
                    del pending_ags[:]
                    q, ic0, n_slot, nblk, passes, ws = u
                    it = ip.tile([128, n_slot // 16], I16, tag="it")
                    nc.sync.dma_start(
                        out=it[:, :], in_=idx_d[:, ic0 : ic0 + n_slot // 16]
                    )
                    npass = len(passes)
                    dt_ = dp.tile([128, npass], BF16, tag="dl")
                    p0 = passes[0][2]
                    nc.sync.dma_start(
                        out=dt_[:, :], in_=drel_d[:, p0 : p0 + npass]
                    )
                    mt = mp.tile([128, p.MAXBLK, 128], BF16, tag="mt")
                    nc.gpsimd.dma_gather(
                        out_ap=mt[:, :nblk, :],
                        in_ap=table[q][:, :],
                        idxs_ap=it[:, :],
                        num_idxs=n_slot,
                        num_idxs_reg=n_slot,
                        elem_size=D,
                        single_packet=False,
                        queue_num=gq_i % max(QROT, 1),
                    )
                    gq_i += 1
                    at = ap_.tile([128, npass, 128], BF16, tag="at")
                    nc.vector.tensor_tensor(
                        at[:, :, :],
                        iota_sb.unsqueeze(1).broadcast_to([128, npass, 128]),
                        dt_.unsqueeze(2).broadcast_to([128, npass, 128]),
                        mybir.AluOpType.is_equal,
                    )
                    psums = {}
                    for b, w, pc, st, sp in passes:
                        if w not in psums:
                            psums[w] = pp.tile(
                                [128, D], F32, tag="ps", name=f"ps{layer}_{q}_{w}"
                            )
                        nc.tensor.matmul(
                            psums[w][:, :],
                            at[:, pc - p0, :],
                            mt[:, b, :],
                            start=st,
                            stop=sp,
                        )
                    for w in sorted(psums):
                        nc.vector.tensor_add(
                            accum[:, w * 128 : (w + 1) * 128],
                            accum[:, w * 128 : (w + 1) * 128],
                            psums[w][:, :],
                        )
                    # once the last chunk's partial sums for this window group
                    # are in, its epilogue can run under the remaining gathers
                    if epi_cb is not None and q == N_CHUNKS - 1:
                        for w in ws:
                            if w < p.real_w and w not in emitted:
                                epi_cb(w)
                                emitted.add(w)
                if epi_cb is not None:
                    for w in range(p.real_w):
                        if w not in emitted:
                            epi_cb(w)
                for ag in ag_ready + pending_ags:
                    ag()
                del ag_ready[:]
                del pending_ags[:]

        def emit_epi1(w, ep, p2):
            q, wq = w // p.WQ, w % p.WQ
            own = ep.tile([128, D], BF16, tag="own")
            nc.sync.dma_start(
                out=own[:, :], in_=gq[0][q][wq * 128 : (wq + 1) * 128, :]
            )
            xw = ep.tile([128, D], F32, tag="xw")
            nc.sync.dma_start(out=xw[:, :], in_=x_tiled[w, :, :])
            t1 = ep.tile([128, D], F32, tag="t1")
            nc.vector.tensor_add(
                t1[:, :], accum[:, w * 128 : (w + 1) * 128], own[:, :]
            )
            nc.vector.tensor_scalar(
                t1[:, :], t1[:, :], dinv_sb[:, w : w + 1], None,
                mybir.AluOpType.mult,
            )
            nc.vector.tensor_add(t1[:, :], t1[:, :], xw[:, :])
            nc.vector.tensor_add(t1[:, :], t1[:, :], b1_sb[:, :])
            h1w = ep.tile([128, D], F32, tag="h1w")
            nc.scalar.activation(
                h1w[:, :], t1[:, :], mybir.ActivationFunctionType.Relu
            )
            nc.sync.dma_start(out=h1_d[w, :, :], in_=h1w[:, :])
            h1b = ep.tile([128, D], BF16, tag="h1b")
            nc.vector.tensor_copy(h1b[:, :], h1w[:, :])
            pt = p2.tile([128, D], BF16, tag="pt")
            nc.tensor.transpose(pt[:, :], h1b[:, :], identb_sb[:, :])
            h1T = ep.tile([128, D], BF16, tag="h1T")
            nc.vector.tensor_copy(h1T[:, :], pt[:, :])
            pg = p2.tile([128, D], F32, tag="pg")
            nc.tensor.matmul(
                pg[:, :], h1T[:, :], w2_sb[:, :], start=True, stop=True
            )
            g2t = ep.tile([128, D], BF16, tag="g2t")
            nc.vector.tensor_scalar(
                g2t[:, :], pg[:, :], dinv_sb[:, w : w + 1], None,
                mybir.AluOpType.mult,
            )
            nc.sync.dma_start(
                out=gq[1][q][wq * 128 : (wq + 1) * 128, :], in_=g2t[:, :]
            )
            if wq == p.WQ - 1 or w == p.real_w - 1:
                if w == p.real_w - 1 and wq != p.WQ - 1:
                    z = ep.tile([128, D], BF16, tag="z")
                    nc.vector.memset(z[:, :], 0.0)
                    for wq2 in range(wq + 1, p.WQ):
                        nc.sync.dma_start(
                            out=gq[1][q][wq2 * 128 : (wq2 + 1) * 128, :],
                            in_=z[:, :],
                        )
                qq = q
                epi1_ag_queue.append(lambda: nc.gpsimd.collective_compute(
                    "AllGather",
                    mybir.AluOpType.bypass,
                    replica_groups=RG,
                    ins=[gq[1][qq][:, :]],
                    outs=[tables[1][qq][:, :]],
                ))

        def emit_epi2(w, ep):
            q, wq = w // p.WQ, w % p.WQ
            own = ep.tile([128, D], BF16, tag="own")
            nc.sync.dma_start(
                out=own[:, :], in_=gq[1][q][wq * 128 : (wq + 1) * 128, :]
            )
            h1w = ep.tile([128, D], F32, tag="h1w")
            nc.sync.dma_start(out=h1w[:, :], in_=h1_d[w, :, :])
            t1 = ep.tile([128, D], F32, tag="t1")
            nc.vector.tensor_add(
                t1[:, :], accum[:, w * 128 : (w + 1) * 128], own[:, :]
            )
            nc.vector.tensor_scalar(
                t1[:, :], t1[:, :], dinv_sb[:, w : w + 1], None,
                mybir.AluOpType.mult,
            )
            nc.vector.tensor_add(t1[:, :], t1[:, :], h1w[:, :])
            nc.vector.tensor_add(t1[:, :], t1[:, :], b2_sb[:, :])
            nc.sync.dma_start(out=out_d[w, :, :], in_=t1[:, :])

        g1_phase()
        with (
            tc.tile_pool(name="e1", bufs=6) as ep1,
            tc.tile_pool(name="e1p", bufs=2, space="PSUM") as p21,
        ):
            agg_phase(1, epi_cb=lambda w: emit_epi1(w, ep1, p21))
        with tc.tile_pool(name="e2", bufs=6) as ep2:
            agg_phase(2, epi_cb=lambda w: emit_epi2(w, ep2))
        ctx.close()

    nc.compile()
    return nc


# --------------------------------------------------------------------------
# Entry point
# --------------------------------------------------------------------------
def kernel(x, edge_index, W1, b1, W2, b2):
    x = np.asarray(x)
    edge_index = np.asarray(edge_index)
    N = x.shape[0]
    p, in_maps = _host_prepare(
        np.asarray(x, dtype=np.float32),
        edge_index,
        np.asarray(W1, dtype=np.float32),
        np.asarray(b1, dtype=np.float32),
        np.asarray(W2, dtype=np.float32),
        np.asarray(b2, dtype=np.float32),
    )
    nc = _build_program(p)

    if os.environ.get("GCN_SIM"):
        from concourse import bass_interp

        sim = bass_interp.MultiCoreSim(nc, N_CORES)
        for c in range(N_CORES):
            for k, v in in_maps[c].items():
                sim.cores[c].tensor(k)[:] = v
        sim.simulate(check_with_hw=False)
        outs = [sim.cores[c].mem_tensor("out") for c in range(N_CORES)]
    else:
        from concourse.bass_utils import run_bass_kernel_spmd

        res = run_bass_kernel_spmd(
            nc,
            in_maps,
            list(range(N_CORES)),
            trace=bool(os.environ.get("GCN_TRACE")),
        )
        kernel.last_result = res
        outs = [res.results[c]["out"] for c in range(N_CORES)]

    full = np.concatenate(
        [
            np.asarray(o, dtype=np.float32).reshape(p.SH, D)[
                : min(p.shard, N - c * p.shard)
            ]
            for c, o in enumerate(outs)
        ],
        axis=0,
    )
    return full.astype(np.float32)
